# revision 9
# baseline (speedup 1.0000x reference)
"""CARAFE upsampler (scale=2, up_kernel=5, enc_kernel=3) on 8 TRN2 NeuronCores.

Sharding: data-parallel over (image n, 16-low-res-row band) -> 8 shards.
Per-core pipeline (all compute on device):
  1) conv1x1 C=256->64 (+bias) on PE, evicted into a width-padded (66-col)
     channel-major layout so the 3x3 conv shifts are pure flat offsets.
  2) 3x3 encoder conv 64->100 as 9 PSUM-accumulating matmuls.
  3) exp() fused into the PSUM eviction (ACT), written in a permuted
     pixel layout pix'' = b*64 + yy*32 + p*4 + xc.
  4) softmax denominators via a constant selector matmul, reciprocal on DVE;
     normalization is applied at the *output* eviction as a per-partition
     scale (per output pixel), so kernels are never normalized explicitly.
  5) reassembly out[m=128 up-pixels, c=256] = W[120,128].T @ featwin[120,256]
     per tile (t = 32 tiles/core), where W is a banded matrix over a
     6x20 low-res window built by 32 SBUF->SBUF DMAs from the exp tensor
     (structural zeros memset once; every band position rewritten each run).
"""

import sys

sys.path.insert(0, "/opt/trn_rl_repo")

from contextlib import ExitStack

import numpy as np

import concourse.bass as bass
import concourse.mybir as mybir
import concourse.tile as tile
from concourse import bacc

F32 = mybir.dt.float32

# problem constants
N_IMG = 2
C = 256
H = W = 64
COMP = 64
ENC = 100  # 25 taps * 4 phases
ROWS = 16  # low-res rows per core
NP_ = 8  # row-pairs per core
XC = 4  # x chunks of 16 bases
NT = NP_ * XC  # 32 tiles per core
KW = 120  # 6 rows * 20 cols window
M = 128  # out pixels per tile: yy*64 + b*4 + ph


def _ap(base, offset, dims):
    """Manual AP on the same tensor (flat element-space steps)."""
    return type(base)(base.tensor, int(offset) + int(base.offset), [list(d) for d in dims])


def build_program():
    nc = bacc.Bacc(None, target_bir_lowering=False)

    fcm_d = nc.dram_tensor("fcm", [256, 18 * 64], F32, kind="ExternalInput")
    fpm_d = nc.dram_tensor("fpm", [20, 68, 256], F32, kind="ExternalInput")
    wcomp_d = nc.dram_tensor("wcomp", [256, 64], F32, kind="ExternalInput")
    bcomp_d = nc.dram_tensor("bcomp", [64, 1], F32, kind="ExternalInput")
    wenc_d = nc.dram_tensor("wenc", [9, 64, 100], F32, kind="ExternalInput")
    sel_d = nc.dram_tensor("sel", [100, 4], F32, kind="ExternalInput")
    out_d = nc.dram_tensor("out", [NT, M, C], F32, kind="ExternalOutput")

    with tile.TileContext(nc) as tc, ExitStack() as ctx:
        const = ctx.enter_context(tc.tile_pool(name="const", bufs=1))
        big = ctx.enter_context(tc.tile_pool(name="big", bufs=1))
        fwin_pool = ctx.enter_context(tc.tile_pool(name="fwin", bufs=3))
        osb_pool = ctx.enter_context(tc.tile_pool(name="osb", bufs=3))
        comp_ps = ctx.enter_context(
            tc.tile_pool(name="comp_ps", bufs=2, space="PSUM")
        )
        kp_ps = ctx.enter_context(tc.tile_pool(name="kp_ps", bufs=2, space="PSUM"))
        s_ps = ctx.enter_context(tc.tile_pool(name="s_ps", bufs=1, space="PSUM"))
        out_ps = ctx.enter_context(tc.tile_pool(name="out_ps", bufs=3, space="PSUM"))

        wcomp_sb = const.tile([128, 2, 64], F32)
        bcomp_sb = const.tile([64, 1], F32)
        wenc_sb = const.tile([64, 9, 100], F32)
        sel_sb = const.tile([100, 4], F32)
        fcm_sb = big.tile([128, 2, 18 * 64], F32)
        comp_sb = big.tile([64, 18, 66], F32)
        exp_sb = big.tile([100, 1024], F32)
        rinv_sb = big.tile([4, 1024], F32)
        rinv_m = big.tile([128, NT], F32)
        expr_sb = big.tile([25, 4096], F32)  # free = yy*2048 + b*128 + ph*32 + t
        w_all = big.tile([KW, M, NT], F32)  # free = m*NT + t

        # ---- input loads ----
        # wcomp [256,64] -> [128, 2, 64], order (p, kc, m); partition dim first
        nc.sync.dma_start(
            _ap(wcomp_sb[:], 0, [[128, 128], [64, 2], [1, 64]]),
            _ap(wcomp_d[:], 0, [[64, 128], [128 * 64, 2], [1, 64]]),
        )
        nc.sync.dma_start(bcomp_sb[:], bcomp_d[:])
        # wenc [9,64,100] -> [64, 9, 100], order (k, tap, m)
        nc.sync.dma_start(
            _ap(wenc_sb[:], 0, [[900, 64], [100, 9], [1, 100]]),
            _ap(wenc_d[:], 0, [[100, 64], [6400, 9], [1, 100]]),
        )
        nc.sync.dma_start(sel_sb[:], sel_d[:])
        # fcm [256, 1152] -> [128, 2, 1152], order (p, kc, flat)
        nc.sync.dma_start(
            _ap(fcm_sb[:], 0, [[2304, 128], [1152, 2], [1, 1152]]),
            _ap(fcm_d[:], 0, [[1152, 128], [128 * 1152, 2], [1, 1152]]),
        )

        # ---- structural zeros (never rewritten; band/valid regions fully
        # rewritten each run) ----
        nc.vector.memset(w_all[:, 0:64, :], 0.0)
        nc.gpsimd.memset(w_all[:, 64:128, :], 0.0)
        nc.vector.memset(comp_sb[:, :, 0:1], 0.0)
        nc.vector.memset(comp_sb[:, :, 65:66], 0.0)

        # ---- conv1x1 + bias -> comp_sb (flat66 channel-major) ----
        for r0, nr in ((0, 8), (8, 8), (16, 2)):
            ps = comp_ps.tile([64, 8 * 64], F32, tag="comp")
            for kc in range(2):
                nc.tensor.matmul(
                    out=ps[:, 0 : nr * 64],
                    lhsT=wcomp_sb[:, kc, :],
                    rhs=fcm_sb[:, kc, r0 * 64 : (r0 + nr) * 64],
                    start=(kc == 0),
                    stop=(kc == 1),
                )
            nc.scalar.activation(
                out=comp_sb[:, r0 : r0 + nr, 1:65],
                in_=_ap(ps[:], 0, [[8 * 64, 64], [64, nr], [1, 64]]),
                func=mybir.ActivationFunctionType.Identity,
                bias=bcomp_sb[:],
                scale=1.0,
            )

        # ---- 3x3 encoder conv -> exp(kp) in pix'' layout ----
        # kp rows r in 0..16 (low-res rows Y0+r), valid x only.
        for r0, nr in ((0, 6), (6, 6), (12, 4)):
            ps = kp_ps.tile([100, 6 * 64], F32, tag="kp")
            for tap in range(9):
                ky, kx = divmod(tap, 3)
                nc.tensor.matmul(
                    out=ps[:, 0 : nr * 64],
                    lhsT=wenc_sb[:, tap, :],
                    rhs=comp_sb[:, r0 + ky : r0 + ky + nr, kx : kx + 64],
                    start=(tap == 0),
                    stop=(tap == 8),
                )
            # evict with exp(), permuting (p', yy, xc, b) -> pix''
            p0 = r0 // 2
            nrp = nr // 2
            for yy in range(2):
                nc.scalar.activation(
                    out=_ap(
                        exp_sb[:],
                        yy * 32 + p0 * 4,
                        [[1024, 100], [64, 16], [4, nrp], [1, 4]],
                    ),
                    in_=_ap(
                        ps[:],
                        yy * 64,
                        [[6 * 64, 100], [1, 16], [128, nrp], [16, 4]],
                    ),
                    func=mybir.ActivationFunctionType.Exp,
                )

        # ---- softmax denominators -> rinv_sb ----
        for k in range(2):
            ps = s_ps.tile([4, 512], F32, tag="s")
            nc.tensor.matmul(
                out=ps[:],
                lhsT=sel_sb[:],
                rhs=exp_sb[:, k * 512 : (k + 1) * 512],
                start=True,
                stop=True,
            )
            nc.vector.reciprocal(rinv_sb[:, k * 512 : (k + 1) * 512], ps[:])

        # ---- rinv_m: per-(tile, out-pixel) scale [128, NT]; m = ph*32+yy*16+b ----
        for yy in range(2):
            for ph in range(4):
                nc.sync.dma_start(
                    _ap(rinv_m[:], (ph * 32 + yy * 16) * NT, [[NT, 16], [1, 32]]),
                    _ap(
                        rinv_sb[:],
                        ph * 1024 + yy * 32,
                        [[1024, 1], [64, 16], [1, 32]],
                    ),
                )

        # ---- W build stage 1: restage exp into expR[tap, (yy,b,ph,t)].
        # c_enc is host-permuted to ph*25+tap so src slices are contiguous
        # partitions. ----
        for yy in range(2):
            for ph in range(4):
                nc.gpsimd.dma_start(
                    _ap(
                        expr_sb[:],
                        yy * 2048 + ph * 32,
                        [[4096, 25], [128, 16], [1, 32]],
                    ),
                    _ap(
                        exp_sb[:],
                        ph * 25 * 1024 + yy * 32,
                        [[1024, 25], [64, 16], [1, 32]],
                    ),
                )

        # ---- W build stage 2: banded placement, one DMA per (yy, b, i);
        # all dims single-partition-stride. ----
        for yy in range(2):
            for b in range(16):
                for i in range(5):
                    nc.gpsimd.dma_start(
                        _ap(
                            w_all[:],
                            (20 * (yy + i) + b) * (M * NT) + (yy * 16 + b) * NT,
                            [[M * NT, 5], [32 * NT, 4], [1, NT]],
                        ),
                        _ap(
                            expr_sb[:],
                            i * 5 * 4096 + yy * 2048 + b * 128,
                            [[4096, 5], [32, 4], [1, 32]],
                        ),
                    )

        # ---- reassembly: 32 tiles ----
        for t in range(NT):
            p, xc = divmod(t, XC)
            fwin = fwin_pool.tile([KW, C], F32, tag="fwin")
            nc.sync.dma_start(
                fwin[:],
                fpm_d[2 * p : 2 * p + 6, 16 * xc : 16 * xc + 20, :],
            )
            ps = out_ps.tile([M, C], F32, tag="out")
            nc.tensor.matmul(
                out=ps[:],
                lhsT=_ap(w_all[:], t, [[M * NT, KW], [NT, M]]),
                rhs=fwin[:],
                start=True,
                stop=True,
            )
            osb = osb_pool.tile([M, C], F32, tag="osb")
            if t % 2 == 0:
                nc.vector.tensor_scalar_mul(osb[:], ps[:], rinv_m[:, t : t + 1])
            else:
                nc.scalar.mul(osb[:], ps[:], rinv_m[:, t : t + 1])
            nc.sync.dma_start(out_d[t], osb[:])

    nc.compile()
    return nc


_CACHE = {}


def _get_program():
    if "nc" not in _CACHE:
        _CACHE["nc"] = build_program()
    return _CACHE["nc"]


def make_in_maps(features, w_comp, b_comp, w_enc):
    features = np.asarray(features, dtype=np.float32)
    w_comp = np.asarray(w_comp, dtype=np.float32)
    b_comp = np.asarray(b_comp, dtype=np.float32)
    w_enc = np.asarray(w_enc, dtype=np.float32)

    # channel-major, rows padded by 1 (for the 3x3 conv halo)
    fpad = np.pad(features, ((0, 0), (0, 0), (1, 1), (0, 0)))  # [2,256,66,64]
    # pixel-major, padded by 2 (reassembly window halo)
    fpm_full = np.pad(
        np.ascontiguousarray(features.transpose(0, 2, 3, 1)),
        ((0, 0), (2, 2), (2, 2), (0, 0)),
    )  # [2,68,68,256]

    wcomp = np.ascontiguousarray(w_comp[:, :, 0, 0].T)  # [256,64]
    bcomp = np.ascontiguousarray(b_comp[:, None])  # [64,1]
    # permute encoder output channels to c_enc = ph*25 + tap
    perm = np.array(
        [tap * 4 + ph for ph in range(4) for tap in range(25)], dtype=np.int64
    )
    wenc = np.ascontiguousarray(
        w_enc[perm].transpose(2, 3, 1, 0).reshape(9, 64, 100)
    )
    sel = np.zeros((100, 4), dtype=np.float32)
    sel[np.arange(100), np.arange(100) // 25] = 1.0

    in_maps = []
    for core in range(8):
        n, band = divmod(core, 4)
        y0 = band * ROWS
        fcm = np.ascontiguousarray(
            fpad[n, :, y0 : y0 + 18, :].reshape(256, 18 * 64)
        )
        fpm = np.ascontiguousarray(fpm_full[n, y0 : y0 + 20, :, :])
        in_maps.append(
            {
                "fcm": fcm,
                "fpm": fpm,
                "wcomp": wcomp,
                "bcomp": bcomp,
                "wenc": wenc,
                "sel": sel,
            }
        )
    return in_maps


def assemble_output(core_outs):
    """core_outs: list of 8 arrays [NT, M, C] -> [2, 256, 128, 128]."""
    final = np.empty((N_IMG, C, 2 * H, 2 * W), dtype=np.float32)
    fv = final.reshape(N_IMG, C, H, 2, W, 2)
    for core, o in enumerate(core_outs):
        n, band = divmod(core, 4)
        y0 = band * ROWS
        # m = ph*32 + yy*16 + b, ph = py*2 + px
        o7 = o.reshape(NP_, XC, 2, 2, 2, 16, C)  # p, xc, py, px, yy, b, c
        # -> c, (p yy), py, (xc b), px
        band_out = o7.transpose(6, 0, 4, 2, 1, 5, 3).reshape(C, ROWS, 2, W, 2)
        fv[n, :, y0 : y0 + ROWS] = band_out
    return final


def kernel(features, w_comp, b_comp, w_enc):
    from concourse.bass_utils import run_bass_kernel_spmd

    nc = _get_program()
    in_maps = make_in_maps(features, w_comp, b_comp, w_enc)
    res = run_bass_kernel_spmd(nc, in_maps, core_ids=list(range(8)))
    return assemble_output([r["out"] for r in res.results])


# revision 10
# speedup vs baseline: 1.0110x; 1.0110x over previous
"""CARAFE upsampler (scale=2, up_kernel=5, enc_kernel=3) on 8 TRN2 NeuronCores.

Sharding: data-parallel over (image n, 16-low-res-row band) -> 8 shards.
Per-core pipeline (all compute on device):
  1) conv1x1 C=256->64 (+bias) on PE, evicted into a width-padded (66-col)
     channel-major layout so the 3x3 conv shifts are pure flat offsets.
  2) 3x3 encoder conv 64->100 as 9 PSUM-accumulating matmuls.
  3) exp() fused into the PSUM eviction (ACT), written in a permuted
     pixel layout pix'' = b*64 + yy*32 + p*4 + xc.
  4) softmax denominators via a constant selector matmul, reciprocal on DVE;
     normalization is applied at the *output* eviction as a per-partition
     scale (per output pixel), so kernels are never normalized explicitly.
  5) reassembly out[m=128 up-pixels, c=256] = W[120,128].T @ featwin[120,256]
     per tile (t = 32 tiles/core), where W is a banded matrix over a
     6x20 low-res window built by 32 SBUF->SBUF DMAs from the exp tensor
     (structural zeros memset once; every band position rewritten each run).
"""

import sys

sys.path.insert(0, "/opt/trn_rl_repo")

from contextlib import ExitStack

import numpy as np

import concourse.bass as bass
import concourse.mybir as mybir
import concourse.tile as tile
from concourse import bacc

F32 = mybir.dt.float32

# problem constants
N_IMG = 2
C = 256
H = W = 64
COMP = 64
ENC = 100  # 25 taps * 4 phases
ROWS = 16  # low-res rows per core
NP_ = 8  # row-pairs per core
XC = 4  # x chunks of 16 bases
NT = NP_ * XC  # 32 tiles per core
KW = 120  # 6 rows * 20 cols window
M = 128  # out pixels per tile: yy*64 + b*4 + ph


def _ap(base, offset, dims):
    """Manual AP on the same tensor (flat element-space steps)."""
    return type(base)(base.tensor, int(offset) + int(base.offset), [list(d) for d in dims])


def build_program():
    nc = bacc.Bacc(None, target_bir_lowering=False)

    fcm_d = nc.dram_tensor("fcm", [256, 18 * 64], F32, kind="ExternalInput")
    fpm_d = nc.dram_tensor("fpm", [20, 68, 256], F32, kind="ExternalInput")
    wcomp_d = nc.dram_tensor("wcomp", [256, 64], F32, kind="ExternalInput")
    bcomp_d = nc.dram_tensor("bcomp", [64, 1], F32, kind="ExternalInput")
    wenc_d = nc.dram_tensor("wenc", [9, 64, 100], F32, kind="ExternalInput")
    sel_d = nc.dram_tensor("sel", [100, 4], F32, kind="ExternalInput")
    out_d = nc.dram_tensor("out", [NT, M, C], F32, kind="ExternalOutput")

    with tile.TileContext(nc) as tc, ExitStack() as ctx:
        const = ctx.enter_context(tc.tile_pool(name="const", bufs=1))
        big = ctx.enter_context(tc.tile_pool(name="big", bufs=1))
        fwin_pool = ctx.enter_context(tc.tile_pool(name="fwin", bufs=3))
        osb_pool = ctx.enter_context(tc.tile_pool(name="osb", bufs=3))
        comp_ps = ctx.enter_context(
            tc.tile_pool(name="comp_ps", bufs=2, space="PSUM")
        )
        kp_ps = ctx.enter_context(tc.tile_pool(name="kp_ps", bufs=2, space="PSUM"))
        s_ps = ctx.enter_context(tc.tile_pool(name="s_ps", bufs=1, space="PSUM"))
        out_ps = ctx.enter_context(tc.tile_pool(name="out_ps", bufs=3, space="PSUM"))

        wcomp_sb = const.tile([128, 2, 64], F32)
        bcomp_sb = const.tile([64, 1], F32)
        wenc_sb = const.tile([64, 9, 100], F32)
        sel_sb = const.tile([100, 4], F32)
        fcm_sb = big.tile([128, 2, 18 * 64], F32)
        comp_sb = big.tile([64, 18, 66], F32)
        exp_sb = big.tile([100, 1024], F32)
        rinv_sb = big.tile([4, 1024], F32)
        rinv_m = big.tile([128, NT], F32)
        expr_sb = big.tile([25, 4096], F32)  # free = yy*2048 + b*128 + ph*32 + t
        w_all = big.tile([KW, M, NT], F32)  # free = m*NT + t

        # ---- input loads ----
        # wcomp [256,64] -> [128, 2, 64], order (p, kc, m); partition dim first
        nc.sync.dma_start(
            _ap(wcomp_sb[:], 0, [[128, 128], [64, 2], [1, 64]]),
            _ap(wcomp_d[:], 0, [[64, 128], [128 * 64, 2], [1, 64]]),
        )
        nc.sync.dma_start(bcomp_sb[:], bcomp_d[:])
        # wenc [9,64,100] -> [64, 9, 100], order (k, tap, m)
        nc.sync.dma_start(
            _ap(wenc_sb[:], 0, [[900, 64], [100, 9], [1, 100]]),
            _ap(wenc_d[:], 0, [[100, 64], [6400, 9], [1, 100]]),
        )
        nc.sync.dma_start(sel_sb[:], sel_d[:])
        # fcm [256, 1152] -> [128, 2, 1152], order (p, kc, flat)
        nc.sync.dma_start(
            _ap(fcm_sb[:], 0, [[2304, 128], [1152, 2], [1, 1152]]),
            _ap(fcm_d[:], 0, [[1152, 128], [128 * 1152, 2], [1, 1152]]),
        )

        # ---- structural zeros (never rewritten; band/valid regions fully
        # rewritten each run) ----
        nc.vector.memset(w_all[:, 0:64, :], 0.0)
        nc.gpsimd.memset(w_all[:, 64:128, :], 0.0)
        nc.vector.memset(comp_sb[:, :, 0:1], 0.0)
        nc.vector.memset(comp_sb[:, :, 65:66], 0.0)

        # ---- conv1x1 + bias -> comp_sb (flat66 channel-major) ----
        for r0, nr in ((0, 8), (8, 8), (16, 2)):
            ps = comp_ps.tile([64, 8 * 64], F32, tag="comp")
            for kc in range(2):
                nc.tensor.matmul(
                    out=ps[:, 0 : nr * 64],
                    lhsT=wcomp_sb[:, kc, :],
                    rhs=fcm_sb[:, kc, r0 * 64 : (r0 + nr) * 64],
                    start=(kc == 0),
                    stop=(kc == 1),
                )
            nc.scalar.activation(
                out=comp_sb[:, r0 : r0 + nr, 1:65],
                in_=_ap(ps[:], 0, [[8 * 64, 64], [64, nr], [1, 64]]),
                func=mybir.ActivationFunctionType.Identity,
                bias=bcomp_sb[:],
                scale=1.0,
            )

        # ---- 3x3 encoder conv -> exp(kp) in pix'' layout ----
        # kp rows r in 0..16 (low-res rows Y0+r), valid x only.
        for r0, nr in ((0, 6), (6, 6), (12, 4)):
            ps = kp_ps.tile([100, 6 * 64], F32, tag="kp")
            for tap in range(9):
                ky, kx = divmod(tap, 3)
                nc.tensor.matmul(
                    out=ps[:, 0 : nr * 64],
                    lhsT=wenc_sb[:, tap, :],
                    rhs=comp_sb[:, r0 + ky : r0 + ky + nr, kx : kx + 64],
                    start=(tap == 0),
                    stop=(tap == 8),
                )
            # evict with exp(), permuting (p', yy, xc, b) -> pix''
            p0 = r0 // 2
            nrp = nr // 2
            for yy in range(2):
                nc.scalar.activation(
                    out=_ap(
                        exp_sb[:],
                        yy * 32 + p0 * 4,
                        [[1024, 100], [64, 16], [4, nrp], [1, 4]],
                    ),
                    in_=_ap(
                        ps[:],
                        yy * 64,
                        [[6 * 64, 100], [1, 16], [128, nrp], [16, 4]],
                    ),
                    func=mybir.ActivationFunctionType.Exp,
                )

        # ---- softmax denominators -> rinv_sb ----
        for k in range(2):
            ps = s_ps.tile([4, 512], F32, tag="s")
            nc.tensor.matmul(
                out=ps[:],
                lhsT=sel_sb[:],
                rhs=exp_sb[:, k * 512 : (k + 1) * 512],
                start=True,
                stop=True,
            )
            nc.vector.reciprocal(rinv_sb[:, k * 512 : (k + 1) * 512], ps[:])

        # ---- rinv_m: per-(tile, out-pixel) scale [128, NT]; m = ph*32+yy*16+b ----
        for yy in range(2):
            for ph in range(4):
                nc.sync.dma_start(
                    _ap(rinv_m[:], (ph * 32 + yy * 16) * NT, [[NT, 16], [1, 32]]),
                    _ap(
                        rinv_sb[:],
                        ph * 1024 + yy * 32,
                        [[1024, 1], [64, 16], [1, 32]],
                    ),
                )

        # ---- W build stage 1: restage exp into expR[tap, (yy,b,ph,t)].
        # c_enc is host-permuted to ph*25+tap so src slices are contiguous
        # partitions. ----
        for yy in range(2):
            for ph in range(4):
                nc.gpsimd.dma_start(
                    _ap(
                        expr_sb[:],
                        yy * 2048 + ph * 32,
                        [[4096, 25], [128, 16], [1, 32]],
                    ),
                    _ap(
                        exp_sb[:],
                        ph * 25 * 1024 + yy * 32,
                        [[1024, 25], [64, 16], [1, 32]],
                    ),
                )

        # ---- W build stage 2: banded placement, one DMA per (yy, b, i);
        # all dims single-partition-stride. ----
        for yy in range(2):
            for b in range(16):
                for i in range(5):
                    nc.gpsimd.dma_start(
                        _ap(
                            w_all[:],
                            (20 * (yy + i) + b) * (M * NT) + (yy * 16 + b) * NT,
                            [[M * NT, 5], [32 * NT, 4], [1, NT]],
                        ),
                        _ap(
                            expr_sb[:],
                            i * 5 * 4096 + yy * 2048 + b * 128,
                            [[4096, 5], [32, 4], [1, 32]],
                        ),
                    )

        # ---- reassembly: 32 tiles ----
        for t in range(NT):
            p, xc = divmod(t, XC)
            fwin = fwin_pool.tile([KW, C], F32, tag="fwin")
            nc.sync.dma_start(
                fwin[:],
                fpm_d[2 * p : 2 * p + 6, 16 * xc : 16 * xc + 20, :],
            )
            ps = out_ps.tile([M, C], F32, tag="out")
            nc.tensor.matmul(
                out=ps[:],
                lhsT=_ap(w_all[:], t, [[M * NT, KW], [NT, M]]),
                rhs=fwin[:],
                start=True,
                stop=True,
            )
            osb = osb_pool.tile([M, C], F32, tag="osb")
            if t % 2 == 0:
                nc.vector.tensor_scalar_mul(osb[:], ps[:], rinv_m[:, t : t + 1])
            else:
                nc.scalar.mul(osb[:], ps[:], rinv_m[:, t : t + 1])
            nc.sync.dma_start(out_d[t], osb[:])

    nc.compile()
    return nc


_CACHE = {}


def _get_program():
    if "nc" not in _CACHE:
        _CACHE["nc"] = build_program()
    return _CACHE["nc"]


def make_in_maps(features, w_comp, b_comp, w_enc):
    features = np.asarray(features, dtype=np.float32)
    w_comp = np.asarray(w_comp, dtype=np.float32)
    b_comp = np.asarray(b_comp, dtype=np.float32)
    w_enc = np.asarray(w_enc, dtype=np.float32)

    # channel-major, rows padded by 1 (for the 3x3 conv halo)
    fpad = np.pad(features, ((0, 0), (0, 0), (1, 1), (0, 0)))  # [2,256,66,64]
    # pixel-major, padded by 2 (reassembly window halo)
    fpm_full = np.pad(
        np.ascontiguousarray(features.transpose(0, 2, 3, 1)),
        ((0, 0), (2, 2), (2, 2), (0, 0)),
    )  # [2,68,68,256]

    wcomp = np.ascontiguousarray(w_comp[:, :, 0, 0].T)  # [256,64]
    bcomp = np.ascontiguousarray(b_comp[:, None])  # [64,1]
    # permute encoder output channels to c_enc = ph*25 + tap
    perm = np.array(
        [tap * 4 + ph for ph in range(4) for tap in range(25)], dtype=np.int64
    )
    wenc = np.ascontiguousarray(
        w_enc[perm].transpose(2, 3, 1, 0).reshape(9, 64, 100)
    )
    sel = np.zeros((100, 4), dtype=np.float32)
    sel[np.arange(100), np.arange(100) // 25] = 1.0

    in_maps = []
    for core in range(8):
        n, band = divmod(core, 4)
        y0 = band * ROWS
        fcm = np.ascontiguousarray(
            fpad[n, :, y0 : y0 + 18, :].reshape(256, 18 * 64)
        )
        fpm = np.ascontiguousarray(fpm_full[n, y0 : y0 + 20, :, :])
        in_maps.append(
            {
                "fcm": fcm,
                "fpm": fpm,
                "wcomp": wcomp,
                "bcomp": bcomp,
                "wenc": wenc,
                "sel": sel,
            }
        )
    return in_maps


def assemble_output(core_outs):
    """core_outs: list of 8 arrays [NT, M, C] -> [2, 256, 128, 128]."""
    final = np.empty((N_IMG, C, 2 * H, 2 * W), dtype=np.float32)
    fv = final.reshape(N_IMG, C, H, 2, W, 2)
    for core, o in enumerate(core_outs):
        n, band = divmod(core, 4)
        y0 = band * ROWS
        # m = ph*32 + yy*16 + b, ph = py*2 + px
        o7 = o.reshape(NP_, XC, 2, 2, 2, 16, C)  # p, xc, py, px, yy, b, c
        # -> c, (p yy), py, (xc b), px
        band_out = o7.transpose(6, 0, 4, 2, 1, 5, 3).reshape(C, ROWS, 2, W, 2)
        fv[n, :, y0 : y0 + ROWS] = band_out
    return final


def _install_neff_cache():
    """Memoize the walrus BIR->NEFF compile on disk (it takes minutes)."""
    if _CACHE.get("neff_cache"):
        return
    import hashlib
    import os
    import shutil

    import concourse.bass_utils as bu
    import concourse.bass2jax as b2j

    cache_dir = "/tmp/bass_neff_cache"
    os.makedirs(cache_dir, exist_ok=True)
    orig = bu.compile_bir_kernel

    def cached(bir_bytes, tmpdir, neff_name="file.neff"):
        h = hashlib.sha256(bir_bytes).hexdigest()[:24]
        cpath = os.path.join(cache_dir, f"{h}.neff")
        dst = os.path.join(tmpdir, neff_name)
        if os.path.exists(cpath):
            shutil.copy(cpath, dst)
            return dst
        out = orig(bir_bytes, tmpdir, neff_name)
        try:
            shutil.copy(out, cpath)
        except OSError:
            pass
        return out

    bu.compile_bir_kernel = cached
    for mod in (b2j,):
        if hasattr(mod, "compile_bir_kernel"):
            mod.compile_bir_kernel = cached
    _CACHE["neff_cache"] = True


def kernel(features, w_comp, b_comp, w_enc):
    from concourse.bass_utils import run_bass_kernel_spmd

    _install_neff_cache()
    nc = _get_program()
    in_maps = make_in_maps(features, w_comp, b_comp, w_enc)
    res = run_bass_kernel_spmd(nc, in_maps, core_ids=list(range(8)))
    return assemble_output([r["out"] for r in res.results])
